# revision 53
# baseline (speedup 1.0000x reference)
"""Trainium2 Bass kernel for nn_CapsuleLayer (dynamic routing capsule layer).

Reference computation (per batch element b):
    u_hat[b,r,c,o] = sum_i W[r,c,o,i] * x[b,r,i]        (R=1152, C=10, O=16, I=8)
    b_ij = 0
    3 routing iterations:
        c_ij = softmax(b_ij, axis=r)
        s_j[c,o] = sum_r c_ij[r,c] * u_hat[r,c,o]
        v = squash(s_j)  over o
        b_ij += sum_o u_hat[r,c,o] * v[c,o]   (except last iteration)
    output v -> [B, 1, C, O, 1]

Sharding: data-parallel over batch B=256 across 8 cores (32 each), W replicated.

Layouts (host prepacks; partition index p = 8*rh + i; r = 16*ch + rh):
  - Df[p, ch, (b, rh')] : block-diag x, columns b-major.
  - build per chunk: pa = Wp[:,ch,0:128].T @ Df -> psum [co(c<8), (b, rh')];
    tail: 4 bq-sliced matmuls Wp[:,ch,128:160].T @ Df[:, 128bq:128bq+128]
    -> one psum [32bq+(cc,o), (bg, rh')] == U1b4 layout for the chunk, so the
    tail drains in ONE [128,128] copy (vs 4 tiny ones).
    s0 (iteration-0 s_j) accumulates in the same phase-A loop:
    xp[:,ch,:].T @ Wp[:,ch,:] -> psum [b, co], summed over all 72 chunks.
  - U1a[16c+o (c<8), b, r] fp16; U1b4[32bq+16cc+o, bg, r] fp16 (bq=b//8,
    bg=b%8).  Drains split ACT/DVE by b-range to balance engine load.
  - b_ij update r-major (per b, rc): lhsT = U1a[:, b, rc] (FWL), rhs = 8-col
    slice of block-diag(v) Vm1; tail batches 4 b's via U1b4/Vm24. One
    [128, 384] J-layout psum per rc accumulated into bsb by DVE adds.
  - softmax: per-rc ACT Exp bsb -> e_stage bf16 (logits stay < 40); Z via
    [128,1]-ones matmul on e_stage -> pz [1, 320].
  - s_j: exp values round-trip through DRAM: e_stage -> E_h[rc] (dram,
    [rh, q, col]) -> broadcast-expanding DMA read -> yce[p=(8rh+i), q, col]
    (each exp row replicated to the 8 i-partitions).  This replaces the old
    EM64 expand-matmul + fp32-psum mul: yc = yce * xp is now a bf16*bf16 DVE
    mul at 2x rate.  Then two 128-col FWL matmuls Wq[:,ch,0:80] @ yc[:,0:160]
    and Wq[:,ch,80:160] @ yc[:,160:320] accumulate the c-diagonal into one
    [80, 320] psum.
  - squash in [b, (c, o)] layout: 10 tiny PE transposes pull the s-diagonal
    into pt_s [32, 160]; Z transposed via 10 K=1 matmuls into [32, 10];
    normalize-by-Z-first squash; v transposed back (2 PE transposes
    + 10 copies) only when the next iteration needs Vm; the final v_b
    [32, 160] f32 DMAs straight out.
"""

import sys

if "/opt/trn_rl_repo" not in sys.path:
    sys.path.insert(0, "/opt/trn_rl_repo")

import numpy as np
import ml_dtypes

import concourse.bass as bass
import concourse.mybir as mybir
from concourse import bacc
from concourse.tile import TileContext

BF16 = mybir.dt.bfloat16
F16 = mybir.dt.float16
F32 = mybir.dt.float32
NPBF16 = ml_dtypes.bfloat16

B, R, C, O, I = 256, 1152, 10, 16, 8
NCORES = 8
BC = B // NCORES          # 32 batch elements per core
CH = R // 16              # 72 chunks of 16 r's
DGRP = 4                  # Df chunks per DMA
EPS = 1e-7
ACT_B = 18                # U1a drain split: b < ACT_B on ACT, rest on DVE
ExpF = mybir.ActivationFunctionType.Exp
SqrtF = mybir.ActivationFunctionType.Sqrt
ADD = mybir.AluOpType.add
AX_X = mybir.AxisListType.X


def _host_prep(xs, W):
    """Per-core input arrays. xs: [32,1152,8] f32, W: [1152,10,16,8] f32."""
    Wr = W.reshape(CH, 16, C, O, I).transpose(1, 4, 0, 2, 3)  # rh,i,ch,c,o
    Wp = Wr.reshape(128, CH * 160).astype(np.float16)
    Wq = Wr.reshape(128, CH * 160).astype(NPBF16)
    xr = xs.reshape(BC, CH, 16, I).transpose(2, 3, 1, 0)  # rh, i, ch, b
    xp = xr.reshape(128, CH * BC).astype(np.float16)
    xpb = xr.reshape(128, CH * BC).astype(NPBF16)
    tmp = xs.reshape(BC, CH, 16, I).transpose(1, 2, 3, 0)  # ch, rh, i, b
    D6 = np.zeros((CH, 16, I, BC, 16), np.float32)         # cols (b, rh')
    for rh in range(16):
        D6[:, rh, :, :, rh] = tmp[:, rh, :, :]
    Df = (D6.reshape(CH, 128, BC * 16).transpose(1, 0, 2)  # [128, CH, 512]
          .astype(np.float16))
    Df = np.ascontiguousarray(Df)
    return {"Wp": Wp, "Wq": Wq, "xp": xp, "xpb": xpb, "Df": Df}


def _host_consts():
    p = np.arange(128)
    # EXP16[o', 16c+o] = (o'==o): replicate vD rows to all capsule rows
    EXP16 = np.tile(np.eye(16, dtype=np.float32), (1, 8)).astype(np.float16)
    # EXP24[o', 32bq+16cc+o] = (o'==o)
    EXP24 = np.tile(np.tile(np.eye(16, dtype=np.float32), (1, 2)),
                    (1, 4)).astype(np.float16)
    # MASK1[16c+o, c'*32+b] = (c==c') for c' < 8
    c_of = (p // 16)[:, None]
    cols = np.arange(256)[None, :] // 32
    MASK1 = (c_of == cols).astype(np.float16)
    # MASK24[32bq+16cc+o, bg*8+bq'*2+cc'] = (bq==bq') & (cc==cc')
    bq_of = (p // 32)[:, None]
    cc_of = ((p % 32) // 16)[:, None]
    col24 = np.arange(64)[None, :]
    MASK24 = ((bq_of == (col24 % 8) // 2) & (cc_of == col24 % 2)).astype(
        np.float16)
    ID32F = np.eye(32, dtype=np.float32)
    ID128F = np.eye(128, dtype=np.float32)
    return {"EXP16": EXP16, "EXP24": EXP24, "MASK1": MASK1,
            "MASK24": MASK24, "ID32F": ID32F, "ID128F": ID128F}


def build_nc(stop_after=2):
    nc = bacc.Bacc("TRN2", target_bir_lowering=False, debug=False,
                   num_devices=NCORES)
    dr = {}
    for name, shape, dt in [
        ("Wp", [128, CH * 160], F16), ("Wq", [128, CH * 160], BF16),
        ("xp", [128, CH * BC], F16), ("xpb", [128, CH * BC], BF16),
        ("Df", [128, CH, 16 * BC], F16),
        ("EXP16", [16, 128], F16), ("EXP24", [16, 128], F16),
        ("MASK1", [128, 256], F16), ("MASK24", [128, 64], F16),
        ("ID32F", [32, 32], F32), ("ID128F", [128, 128], F32),
    ]:
        dr[name] = nc.dram_tensor(name, shape, dt, kind="ExternalInput").ap()
    d_out = nc.dram_tensor("out", [BC, 160], F32, kind="ExternalOutput").ap()

    with TileContext(nc) as tc:
        _emit(nc, tc, dr, d_out, stop_after)
    nc.compile()
    return nc


def _emit(nc, tc, dr, d_out, stop_after=2):
    from contextlib import ExitStack

    with ExitStack() as ctx:
        consts = ctx.enter_context(tc.tile_pool(name="consts", bufs=1))
        upool = ctx.enter_context(tc.tile_pool(name="upool", bufs=1))
        bpool = ctx.enter_context(tc.tile_pool(name="bpool", bufs=1))
        estream = ctx.enter_context(tc.tile_pool(name="estream", bufs=3))
        ystream = ctx.enter_context(tc.tile_pool(name="ystream", bufs=3))
        ehpool = ctx.enter_context(tc.tile_pool(name="ehpool", bufs=10,
                                                space="DRAM"))
        # build-phase-only SBUF (scoped: freed before yce allocation)
        bld_ctx = ExitStack()
        bldpool = bld_ctx.enter_context(tc.tile_pool(name="bldpool", bufs=1))
        dstream = bld_ctx.enter_context(tc.tile_pool(name="dstream", bufs=3))

        # ---- resident tiles ----
        Wp = bldpool.tile([128, CH, 160], F16)
        Wq = consts.tile([128, CH, 160], BF16)
        xp = consts.tile([128, CH, BC], F16)
        xpb = consts.tile([128, CH, BC], BF16)
        EXP16 = consts.tile([16, 128], F16)
        EXP24 = consts.tile([16, 128], F16)
        MASK1 = consts.tile([128, 256], F16)
        MASK24 = consts.tile([128, 64], F16)
        ID32F = consts.tile([32, 32], F32)
        ID128F = consts.tile([128, 128], F32)
        onescol = consts.tile([128, 1], BF16)
        dumm = consts.tile([1, 1], F32)
        cEPS = consts.tile([32, 1], F32)    # squash epsilon

        # DMA priority order on the sync queue: xp + first half of Wp gate
        # phase A; small consts gate fill_vm(v0); Wq/xpb not needed till s_j.
        nc.sync.dma_start(out=xp[:].rearrange("p a b -> p (a b)"), in_=dr["xp"])
        Wp2 = Wp[:].rearrange("p a b -> p (a b)")
        H = (CH // 4) * 160
        for k in range(4):
            nc.sync.dma_start(out=Wp2[:, k * H:(k + 1) * H],
                              in_=dr["Wp"][:, k * H:(k + 1) * H])
        for nm, t in [("EXP16", EXP16), ("EXP24", EXP24), ("MASK1", MASK1),
                      ("MASK24", MASK24), ("ID32F", ID32F),
                      ("ID128F", ID128F)]:
            nc.sync.dma_start(out=t[:], in_=dr[nm])
        nc.gpsimd.memset(onescol[:], 1.0)
        nc.gpsimd.memset(dumm[:], 1.0)
        nc.gpsimd.memset(cEPS[:], EPS)

        U1a = upool.tile([128, BC, R], F16)     # [16c+o (c<8), b, r]
        U1b4 = upool.tile([128, 8, R], F16)     # [32bq+16cc+o, bg, r]

        bsb = bpool.tile([128, 9, 320], F32)    # b_ij r-major, cols J=c*32+b
        Vm1 = bpool.tile([128, 256], F16)       # block-diag v, cols c*32+b
        Vm24 = bpool.tile([128, 64], F16)       # [32bq+16cc+o, (bq',cc',bg)]

        # ---- HAM warm-up: dummy matmuls fill the input-DMA wait so the
        # PE clock is at 8/8 when phase A starts and never re-throttles ----
        warm = consts.tile([128, 512], F16)
        nc.gpsimd.memset(warm[:], 0.0625)
        ppwarm = ctx.enter_context(tc.tile_pool(name="ppwarm", bufs=1,
                                                space="PSUM"))
        pw_holder = [ppwarm.tile([128, 512], F32, tag="pw", name="pw")]

        def dummy(n, cols=384):
            """Warm-keeper matmuls: keep the PE activity monitor at 8/8
            through phases where real PE work is sparse."""
            for _ in range(n):
                nc.tensor.matmul(pw_holder[0][:, 0:cols], warm[:, 0:128],
                                 warm[:, 0:cols], start=True, stop=True,
                                 skip_group_check=True)

        def dummy_on(anchor, n):
            """Warm-keepers anchored to `anchor` (a fresh 2-D SBUF tile) so
            the Tile scheduler cannot float them into PE-dense phases."""
            k, w = anchor.shape[0], min(anchor.shape[1], 320)
            for _ in range(n):
                nc.tensor.matmul(pw_holder[0][0:128, 0:w],
                                 warm[0:k, 0:128], anchor[0:k, 0:w],
                                 start=True, stop=True,
                                 skip_group_check=True)

        dummy(14, 512)

        # ============ Phase A: iteration-0 s (no Df needed) ============
        with tc.tile_pool(name="pps0", bufs=1, space="PSUM") as pp_s0:
            ps0 = pp_s0.tile([32, 160], F32)
            for ch in range(CH):
                nc.tensor.matmul(ps0[:], xp[:, ch, :], Wp[:, ch, :],
                                 start=(ch == 0), stop=(ch == CH - 1),
                                 skip_group_check=True)
            stf0 = bpool.tile([32, 160], F32, tag="stf0")
            nc.scalar.copy(stf0[:], ps0[:])

        # first two Df groups go out on the sync queue ahead of Wq/xpb so
        # the build is never DMA-gated (2-group prefetch depth)
        NG = CH // DGRP
        dfg_tiles = [None] * NG

        def df_fetch(g):
            dfg_tiles[g] = dstream.tile([128, DGRP, 16 * BC], F16, tag="dfg",
                                        name=f"dfg{g}")
            nc.sync.dma_start(
                out=dfg_tiles[g][:].rearrange("p a b -> p (a b)"),
                in_=dr["Df"][:, g * DGRP:(g + 1) * DGRP, :].rearrange(
                    "p a b -> p (a b)"))

        df_fetch(0)
        df_fetch(1)
        # later-needed consts go behind phase A's gating DMAs
        nc.sync.dma_start(out=Wq[:].rearrange("p a b -> p (a b)"),
                          in_=dr["Wq"])
        nc.sync.dma_start(out=xpb[:].rearrange("p a b -> p (a b)"),
                          in_=dr["xpb"])

        # routing-phase PSUM pools
        pp_bb = ctx.enter_context(tc.tile_pool(name="ppup", bufs=2,
                                               space="PSUM"))
        pp_z = ctx.enter_context(tc.tile_pool(name="ppz", bufs=1,
                                              space="PSUM"))
        pp_q_holder = [None, "scr"]

        def squash_b(it, stf_raw, zb):
            """[32, (c, o)] squash.  stf_raw = raw S (f32 SBUF); zb [32, 10]
            f32 SBUF holds Z (None for it=0: Z=R).  Normalizes by Z FIRST --
            the (Z^2+SS)-form denominator overflows fp32 at iteration 2."""
            stf = bpool.tile([32, 160], F32, tag="stf")
            if it == 0:
                nc.vector.tensor_scalar_mul(stf[:], stf_raw[:], 1.0 / R)
            else:
                rz = bpool.tile([32, 10, 1], F32, tag="rz")
                nc.vector.reciprocal(rz[:, :, 0], zb[:])
                nc.vector.tensor_mul(
                    stf[:].rearrange("p (c o) -> p c o", o=16),
                    stf_raw[:].rearrange("p (c o) -> p c o", o=16),
                    rz[:].broadcast_to([32, 10, 16]))
            sq2 = bpool.tile([32, 160], F32, tag="sq2")
            nc.vector.tensor_mul(sq2[:], stf[:], stf[:])
            SS = bpool.tile([32, 10], F32, tag="SS")
            nc.vector.tensor_reduce(
                SS[:], sq2[:].rearrange("p (c o) -> p c o", o=16),
                axis=AX_X, op=ADD)
            r1 = bpool.tile([32, 10], F32, tag="r1")
            nc.vector.tensor_scalar_add(r1[:], SS[:], 1.0)
            r3 = bpool.tile([32, 10], F32, tag="r3")
            nc.vector.tensor_scalar_add(r3[:], SS[:], EPS)
            rt = bpool.tile([32, 10], F32, tag="rt")
            nc.scalar.activation(rt[:], r3[:], SqrtF)
            den = bpool.tile([32, 10], F32, tag="den")
            nc.vector.tensor_mul(den[:], r1[:], rt[:])
            inv = bpool.tile([32, 10], F32, tag="inv")
            nc.vector.reciprocal(inv[:], den[:])
            scl = bpool.tile([32, 10, 1], F32, tag="scl")
            nc.vector.tensor_mul(scl[:, :, 0], SS[:], inv[:])
            v_b = bpool.tile([32, 160], F32, tag="v_b")
            nc.vector.tensor_mul(
                v_b[:].rearrange("p (c o) -> p c o", o=16),
                stf[:].rearrange("p (c o) -> p c o", o=16),
                scl[:].broadcast_to([32, 10, 16]))
            return v_b

        def fill_vm(v_b):
            """v_b [32, 160] f32 -> vD [16, 320] -> Vm1/Vm24 (for b_up)."""
            ptv = pp_q_holder[0].tile([128, 320], F32, tag=pp_q_holder[1])
            for c in range(C):
                nc.tensor.transpose(ptv[0:16, 32 * c:32 * c + 32],
                                    v_b[:, 16 * c:16 * c + 16], ID32F[:])
            vD = bpool.tile([16, 320], F16, tag="vD")
            nc.vector.tensor_copy(vD[:], ptv[0:16, 0:320])
            ppv = pp_q_holder[0].tile([128, 320], F32, tag=pp_q_holder[1])
            nc.tensor.matmul(ppv[0:128, 0:256], EXP16[:], vD[:, 0:256],
                             start=True, stop=True)
            v24 = vD[:, 256:320].rearrange("p (c q g) -> p g q c", c=2, g=8)
            nc.tensor.matmul(ppv[0:128, 256:320], EXP24[:], v24,
                             start=True, stop=True)
            nc.vector.tensor_mul(Vm1[:], ppv[0:128, 0:256], MASK1[:])
            nc.vector.tensor_mul(Vm24[:], ppv[0:128, 256:320], MASK24[:])
            dummy_on(Vm1, 3)

        Vm1v = Vm1[:].rearrange("p (c b) -> p c b", b=BC)       # [128,8,32]

        def b_up_rc(it, rc, pz, eh_list):
            """One r-chunk of the b_ij update + exp + Z + dram stage-out."""
            r0 = 128 * rc
            pbb = pp_bb.tile([128, 384], F32, tag="pbb")
            pbv = pbb[:, 0:320].rearrange("p (c b) -> p c b", b=BC)
            for b in range(BC):
                nc.tensor.matmul(pbv[:, 0:8, b],
                                 U1a[:, b, r0:r0 + 128], Vm1v[:, :, b],
                                 start=True, stop=True)
            for bg in range(8):
                nc.tensor.matmul(
                    pbb[:, 320 + 8 * bg:328 + 8 * bg],
                    U1b4[:, bg, r0:r0 + 128],
                    Vm24[:, 8 * bg:8 * bg + 8],
                    start=True, stop=True)
            # tail cols (bg, bq', cc') -> J-cols 256 + cc'*32 + 8bq' + bg
            h2src = pbb[:, 320:384].rearrange(
                "p (g q c) -> p g q c", q=4, c=2)
            h2dst = bsb[:, rc, 256:320].rearrange(
                "p (c q g) -> p g q c", q=4, g=8)
            if it == 1:
                nc.vector.tensor_copy(bsb[:, rc, 0:256], pbb[:, 0:256])
                nc.vector.tensor_copy(h2dst, h2src)
            else:
                nc.vector.tensor_add(bsb[:, rc, 0:256], bsb[:, rc, 0:256],
                                     pbb[:, 0:256])
                nc.vector.tensor_add(h2dst, h2dst, h2src)
            est = estream.tile([128, 320], BF16, tag="est")
            nc.scalar.activation(est[:], bsb[:, rc, :], ExpF)
            nc.tensor.matmul(pz[:], onescol[:], est[:],
                             start=(rc == 0), stop=(rc == 8),
                             skip_group_check=True)
            # stage exp out to dram in [rh, q, col] order for the
            # broadcast-expanding read-back (est partition p = 16q + rh)
            eh = ehpool.tile([16, 8, 320], BF16, tag="eh", name="eh")
            nc.sync.dma_start(out=eh.rearrange("h q n -> q h n"), in_=est[:])
            eh_list.append(eh)

        # ====== Phase B: build u_hat, fused with iteration-1 b_ij ======
        ehs1 = []
        pz1 = pp_z.tile([1, 320], F32, tag="pz")
        with tc.tile_pool(name="ppba", bufs=2, space="PSUM") as ppb1, \
             tc.tile_pool(name="ppbb", bufs=2, space="PSUM") as ppb2:
            for g in range(NG):
                dfg = dfg_tiles[g]
                if g + 2 < NG:
                    df_fetch(g + 2)
                for j in range(DGRP):
                    ch = g * DGRP + j
                    dfc = dfg[:, j, :]
                    pa = ppb1.tile([128, 16 * BC], F32, tag="pa")
                    pb = ppb2.tile([128, 16 * BC], F32, tag="pb")
                    nc.tensor.matmul(pa[:], Wp[:, ch, 0:128], dfc,
                                     start=True, stop=True)
                    # tail: 4 bq-sliced col-tiled matmuls -> psum already in
                    # U1b4 layout [32bq+(cc,o), (bg, rh)]
                    for bq in range(4):
                        nc.tensor.matmul(
                            pb[32 * bq:32 * bq + 32, 0:128],
                            Wp[:, ch, 128:160],
                            dfc[:, 128 * bq:128 * bq + 128],
                            start=True, stop=True,
                            tile_position=(0, 32 * bq))
                    # psum cols are (b, rh); U1a cols are (b, r=16ch+rh).
                    # Split the big drain ACT/DVE by b-range for balance.
                    pa_v = pa[:].rearrange("p (b h) -> p b h", h=16)
                    nc.scalar.copy(U1a[:, 0:ACT_B, 16 * ch:16 * ch + 16],
                                   pa_v[:, 0:ACT_B, :])
                    nc.vector.tensor_copy(
                        U1a[:, ACT_B:BC, 16 * ch:16 * ch + 16],
                        pa_v[:, ACT_B:BC, :])
                    pb_v = pb[:, 0:128].rearrange("p (g h) -> p g h", h=16)
                    if ch % 2 == 0:
                        nc.vector.tensor_copy(
                            U1b4[:, :, 16 * ch:16 * ch + 16], pb_v)
                    else:
                        nc.scalar.copy(
                            U1b4[:, :, 16 * ch:16 * ch + 16], pb_v)
                    # iteration-1 b_ij for r-chunk rc as soon as its 8
                    # u_hat chunks are drained
                    if ch % 8 == 7:
                        b_up_rc(1, ch // 8, pz1, ehs1)
                # v0 squash+fill emitted after the first group so the PE
                # stream has no gap between phase A and the build (its DVE
                # chain runs under group-0's matmuls; HAM stays warm).
                # Scratch psum borrows pp_bb slots — they are unused until
                # the first b_up_rc at chunk 7, so no build stall.  The
                # guard dummy (anchored on chunk-0's U1a drain) stops the
                # scheduler from queueing the v0 transposes ahead of the
                # first build chunks, which would head-block the PE.
                if g == 0:
                    dummy_on(U1a[:, 0, 0:16], 1)
                    pp_q_holder[0], pp_q_holder[1] = pp_bb, "pbb"
                    v0 = squash_b(0, stf0, None)
                    fill_vm(v0)

        bld_ctx.close()   # free Wp + Df SBUF for the yce stream

        pp_s = ctx.enter_context(tc.tile_pool(name="pps", bufs=1,
                                              space="PSUM"))
        pp_q = ctx.enter_context(tc.tile_pool(name="ppq", bufs=1,
                                              space="PSUM"))
        pp_q_holder[0], pp_q_holder[1] = pp_q, "scr"

        # yce stream allocated after the build pool closes (SBUF reuse)
        ycestream = ctx.enter_context(tc.tile_pool(name="ycestream", bufs=4))

        def yce_fetch(eh_list, rc):
            """Broadcast-expanding readback: yce[8rh+i, q, n] = E[rh, q, n]."""
            yce = ycestream.tile([128, 8, 320], BF16, tag="yce")
            nc.sync.dma_start(
                out=yce[:],
                in_=eh_list[rc].rearrange(
                    "(h x) q n -> h x (q n)", x=1).broadcast_to(
                    [16, 8, 8 * 320]))
            return yce

        def sj_chunk(ch, psd, yce, nw=2):
            """yc = yce * xp (bf16), then the two c-diagonal matmuls."""
            q = ch % 8
            yc = ystream.tile([128, 320], BF16, tag="yc")
            xb = xpb[:, ch:ch + 1, :].broadcast_to([128, 10, BC])
            nc.vector.tensor_mul(
                yc[:].rearrange("p (c b) -> p c b", b=BC),
                yce[:, q, :].rearrange("p (c b) -> p c b", b=BC), xb)
            # start exactly once per bank (a start clears has_written for
            # the WHOLE bank and would orphan earlier columns)
            nc.tensor.matmul(psd[:, 0:160],
                             Wq[:, ch, 0:80], yc[:, 0:160],
                             start=(ch == 0), stop=False,
                             skip_group_check=True)
            nc.tensor.matmul(psd[:, 160:320],
                             Wq[:, ch, 80:160], yc[:, 160:320],
                             start=False, stop=(ch == CH - 1),
                             skip_group_check=True)
            dummy_on(yc, nw)   # hold HAM at 8/8 through the DVE-bound s_j

        def extract_squash(it, psd, pz):
            """s-diagonal + Z -> [32, *] via PE transposes, then squash."""
            nc.scalar.activation(dumm[:], dumm[:], SqrtF)  # preload table
            sfull = bpool.tile([80, 320], F32, tag="sfull")
            nc.scalar.copy(sfull[:], psd[0:80, :])
            zz = bpool.tile([1, 320], F32, tag="zz")
            nc.vector.tensor_copy(zz[:], pz[:])
            pts = pp_q_holder[0].tile([128, 320], F32, tag="scr")
            ptq = pts[0:32, 0:176]
            for c in range(C):
                if c < 5:
                    blk, row = sfull[:, 32 * c:32 * c + 32], 16 * c
                else:
                    blk = sfull[:, 160 + 32 * (c - 5):160 + 32 * (c - 5) + 32]
                    row = 16 * (c - 5)
                nc.tensor.transpose(ptq[:, 16 * c:16 * c + 16], blk,
                                    ID128F[0:80, row:row + 16])
            zz3 = zz[:].rearrange("p (c b) -> p c b", b=BC)
            for c in range(C):
                nc.tensor.matmul(ptq[:, 160 + c:161 + c], zz3[:, c, :],
                                 ID128F[0:1, 0:1], start=True, stop=True)
            stf_b = bpool.tile([32, 160], F32, tag="stf_b")
            nc.vector.tensor_copy(stf_b[:], ptq[:, 0:160])
            zb = bpool.tile([32, 10], F32, tag="zb")
            nc.vector.tensor_copy(zb[:], ptq[:, 160:170])
            # f16 echo (on idle gpsimd) anchors warm-keepers to this phase
            stfh = bpool.tile([32, 160], F16, tag="stfh")
            nc.gpsimd.tensor_copy(stfh[:], stf_b[:])
            dummy_on(stfh, 8)   # squash chain is serial DVE; keep PE warm
            v_b = squash_b(it, stf_b, zb)
            vbh = bpool.tile([32, 160], F16, tag="vbh")
            nc.gpsimd.tensor_copy(vbh[:], v_b[:])
            dummy_on(vbh, 6)
            return v_b

        PF = 3  # yce prefetch depth (rc granularity)

        def sj_iter(psd, eh_list):
            ycet = {}
            for rc in range(PF):
                ycet[rc] = yce_fetch(eh_list, rc)
            for ch in range(CH):
                rc, q = divmod(ch, 8)
                if q == 0 and rc + PF < 9:
                    ycet[rc + PF] = yce_fetch(eh_list, rc + PF)
                sj_chunk(ch, psd, ycet[rc])

        # ============ iteration 1: s_j + squash (b_ij done in build) ======
        psd1 = pp_s.tile([80, 320], F32, tag="psd")
        sj_iter(psd1, ehs1)
        v1 = extract_squash(1, psd1, pz1)
        fill_vm(v1)

        # ============ iteration 2 ============
        if stop_after >= 2:
            ehs2 = []
            psd2 = pp_s.tile([80, 320], F32, tag="psd")
            pz2 = pp_z.tile([1, 320], F32, tag="pz")
            # interleave: emit sj chunks for rc-2 after b_up(rc) so the PE
            # never waits on the exp->dram->yce->yc chain
            ycet = {}
            for rc in range(9):
                b_up_rc(2, rc, pz2, ehs2)
                ycet[rc] = yce_fetch(ehs2, rc)
                if rc >= 2:
                    for q in range(8):
                        sj_chunk(8 * (rc - 2) + q, psd2, ycet[rc - 2], nw=1)
            for rc in (7, 8):
                for q in range(8):
                    sj_chunk(8 * rc + q, psd2, ycet[rc], nw=1)
            v2 = extract_squash(2, psd2, pz2)
            nc.sync.dma_start(out=d_out[:], in_=v2[:])
        else:
            nc.sync.dma_start(out=d_out[:], in_=v1[:])


_NC_CACHE = None


def _get_nc():
    global _NC_CACHE
    if _NC_CACHE is None:
        _NC_CACHE = build_nc()
    return _NC_CACHE


def decode_out(o):
    """[32, 160] core output (b, (c, o)) -> [32, 10, 16] (b, c, o)."""
    return o.reshape(BC, C, O)


def kernel(x, W):
    """Full-input entry point. x: [256,1152,8] f32, W: [1152,10,16,8] f32."""
    from concourse.bass_utils import run_bass_kernel_spmd

    x = np.asarray(x, np.float32)
    W = np.asarray(W, np.float32)
    nc = _get_nc()
    consts = _host_consts()
    in_maps = []
    for k in range(NCORES):
        m = _host_prep(x[k * BC:(k + 1) * BC], W)
        m.update(consts)
        in_maps.append(m)
    res = run_bass_kernel_spmd(nc, in_maps, core_ids=list(range(NCORES)))
    v = np.concatenate([decode_out(res.results[k]["out"])
                        for k in range(NCORES)], axis=0)  # [256, 10, 16]
    return v[:, None, :, :, None].astype(np.float32)
